# revision 5
# baseline (speedup 1.0000x reference)
"""Trainium2 Bass kernel for nn_AnswerModule (scatter_memory, 8 cores).

Strategy: pure data-parallel over batch (4 examples per core).  The
heavy einsums are collapsed via matmul associativity:
  p1 = softmax((s@W6) @ M),  attn@W7b = p1 @ (M^T @ W7b),
  p2 = softmax((s@W7t + attn@W7b) @ M).
The tiny GRU / alpha-attention recurrence runs on host.

This revision (vs the first working kernel) targets three measured
bottlenecks:
  1. DMA descriptor-processing bound (input stream took ~78us at 4-8KB
     descriptors): m/mt now land as 8 fat DMAs with 16KB contiguous
     runs per partition, reaching the ~45us byte-bound floor.
  2. Scalar engine at 48us: EXP activations ran on (4,512) tiles using
     4/128 partitions.  Logit waves for all 4 examples now target one
     (128,512) psum tile via zero-padded 32-col stationaries (junk rows
     compute exp(0)=1, matching the old memset semantics), so each EXP
     act covers 128 rows.  accum_out on the same act yields the softmax
     Z-partials for free (removes the DVE reduy reduces).
  3. PE stalls/p-state: one long phase-ordered PE stream (l1 waves ->
     transposes -> u/cav pipeline with fin1 + pairwise l2 waves filling
     the mt-DMA gaps -> l2 tail -> fin2).
"""

import sys

sys.path.insert(0, "/opt/trn_rl_repo")

import numpy as np
import ml_dtypes

import concourse.bass as bass
import concourse.bacc as bacc
import concourse.mybir as mybir
from concourse import tile
from concourse.bass_utils import run_bass_kernel_spmd

B, QL, PL, T, D2 = 32, 64, 4096, 4, 256
NCORES = 8
BL = B // NCORES  # 4 examples per core
NG = 8  # n-groups of 512
F32 = mybir.dt.float32
F32R = mybir.dt.float32r
BF16 = mybir.dt.bfloat16
FP16 = mybir.dt.float16

_NC = None


def _build_graph():
    nc = bacc.Bacc("TRN2", target_bir_lowering=False, debug=False)

    # DRAM inputs.  m: quarter q covers n-groups 2q,2q+1; col layout
    # 2048*b + 1024*dc + c.  mt: per-example M^T, col 2048*kg + 256*kk + e.
    m_d = nc.dram_tensor("m", [4, 128, 8192], FP16, kind="ExternalInput").ap()
    mt_d = nc.dram_tensor("mt", [BL, 128, 8192], BF16, kind="ExternalInput").ap()
    sw6p_d = nc.dram_tensor("sw6p", [128, 256], FP16, kind="ExternalInput").ap()
    v1t_d = nc.dram_tensor("v1t", [2, 128, 16], FP16, kind="ExternalInput").ap()
    w7b_d = nc.dram_tensor("w7b", [2, 128, 256], F32R, kind="ExternalInput").ap()
    eye_d = nc.dram_tensor("eye", [128, 128], BF16, kind="ExternalInput").ap()
    eye4_d = nc.dram_tensor("eye4", [4, 4], F32R, kind="ExternalInput").ap()
    p4g_d = nc.dram_tensor("p4g", [128, 256], BF16, kind="ExternalInput").ap()
    o1_d = nc.dram_tensor("o1", [BL, PL], F32, kind="ExternalOutput").ap()
    o2_d = nc.dram_tensor("o2", [BL, PL], F32, kind="ExternalOutput").ap()

    EXP = mybir.ActivationFunctionType.Exp
    LOG = getattr(mybir.ActivationFunctionType, "Log", None) or getattr(
        mybir.ActivationFunctionType, "Ln"
    )

    with tile.TileContext(nc) as tc:
        with (
            nc.allow_low_precision(reason="float32r is 4-byte, same width as f32"),
            tc.tile_pool(name="const", bufs=1) as cpool,
            tc.tile_pool(name="m", bufs=1) as mpool,
            tc.tile_pool(name="mt", bufs=1) as mtpool,
            tc.tile_pool(name="sb", bufs=1) as spool,
            tc.tile_pool(name="mega", bufs=3, space="PSUM") as megapool,
            tc.tile_pool(name="pst", bufs=2, space="PSUM") as pstpool,
            tc.tile_pool(name="psu", bufs=2, space="PSUM") as psupool,
            tc.tile_pool(name="sm", bufs=1, space="PSUM") as smpool,
        ):
            # ---- SBUF tiles ----
            sw6p_sb = cpool.tile([128, 256], FP16, tag="sw6p")
            v1t_sb = cpool.tile([128, 32], FP16, tag="v1t")
            w7b_sb = []
            eye_sb = cpool.tile([128, 128], BF16, tag="eye")
            eye4_sb = cpool.tile([4, 4], F32R, tag="eye4")
            p4g_sb = cpool.tile([128, 256], BF16, tag="p4g")
            m_sb = mpool.tile([128, 32768], FP16, tag="m")
            mt_sb = [
                mtpool.tile([128, 8192], BF16, tag="mt", name=f"mt{b}")
                for b in range(BL)
            ]
            exp1_sb = spool.tile([128, PL], BF16, tag="exp1")
            exp2_sb = spool.tile([128, PL], BF16, tag="exp2")
            exp1T = spool.tile([128, PL], BF16, tag="exp1T")
            zp1c = spool.tile([128, NG], F32, tag="zp1c")
            zp2c = spool.tile([128, NG], F32, tag="zp2c")
            v2tp = spool.tile([128, 256], FP16, tag="v2tp")
            uT = spool.tile([128, 32], F32R, tag="uT")

            # ---- small-const DMAs on the scalar HWDGE queue (parallel to
            # the payload queue) except sw6p/eye which ride the sync queue
            # early.  LOG table preload keys off sw6p so it fires ~0.2us in;
            # scale=0 + bias=1 keeps the junk result finite (log(1)=0).
            for dc in range(2):
                t_ = cpool.tile([128, 256], F32R, tag=f"w7b{dc}", name=f"w7b{dc}")
                nc.scalar.dma_start(out=t_[:], in_=w7b_d[dc])
                w7b_sb.append(t_)
            nc.scalar.dma_start(out=p4g_sb[:], in_=p4g_d[:, :])
            nc.scalar.dma_start(out=eye4_sb[:], in_=eye4_d[:, :])
            for dc in range(2):
                nc.scalar.dma_start(
                    out=v1t_sb[:, 16 * dc : 16 * dc + 16], in_=v1t_d[dc]
                )

            # ---- payload DMAs on the sync (SP) queue, priority order ----
            nc.sync.dma_start(out=sw6p_sb[:], in_=sw6p_d[:, :])
            nc.sync.dma_start(out=eye_sb[:], in_=eye_d[:, :])
            for q in range(4):
                nc.sync.dma_start(
                    out=m_sb[:, 8192 * q : 8192 * q + 8192], in_=m_d[q]
                )
            for b in range(BL):
                nc.sync.dma_start(out=mt_sb[b][:], in_=mt_d[b])

            # LOG act-table preload (table also holds EXP -> no later swaps)
            scr = cpool.tile([32, 4], F32, tag="scr")
            nc.scalar.activation(
                scr[:], sw6p_sb[0:32, 0:8].bitcast(F32), LOG, scale=0.0, bias=1.0
            )
            nc.gpsimd.memset(v2tp[:], 0.0)

            def m_slice(b, dc, g):
                off = 8192 * (g // 2) + 2048 * b + 1024 * dc + 512 * (g % 2)
                return m_sb[:, off : off + 512]

            # ---- phase A: l1 logit waves, dense EXP evac + Z partials ----
            def transpose_chunk(k):
                pst = pstpool.tile([128, 128], BF16, tag="pst", name=f"pst{k}")
                nc.tensor.transpose(
                    pst[:], exp1_sb[:, 128 * k : 128 * k + 128], eye_sb[:]
                )
                nc.vector.tensor_copy(exp1T[:, 128 * k : 128 * k + 128], pst[:])

            for g in range(NG):
                mega = megapool.tile([128, 512], F32, tag="mega", name=f"wa{g}")
                for dc in range(2):
                    for b in range(BL):
                        nc.tensor.matmul(
                            mega[32 * b : 32 * b + 32, :],
                            sw6p_sb[:, 128 * dc + 32 * b : 128 * dc + 32 * b + 32],
                            m_slice(b, dc, g),
                            start=(dc == 0),
                            stop=(dc == 1),
                            tile_position=(0, 32 * b),
                        )
                nc.scalar.activation(
                    exp1_sb[:, 512 * g : 512 * g + 512],
                    mega[:],
                    EXP,
                    accum_out=zp1c[:, g : g + 1],
                )
                if g > 0:
                    for j in range(4):
                        transpose_chunk(4 * (g - 1) + j)
            for j in range(4):
                transpose_chunk(28 + j)

            # softmax1 normalizers
            z1 = spool.tile([128, 1], F32, tag="z1")
            nc.vector.tensor_reduce(
                z1[:], zp1c[:], axis=mybir.AxisListType.X, op=mybir.AluOpType.add
            )
            rz1 = spool.tile([128, 1], F32, tag="rz1")
            nc.vector.reciprocal(rz1[:], z1[:])
            bd31 = spool.tile([128, 256], BF16, tag="bd31")
            nc.vector.tensor_scalar_mul(bd31[:], p4g_sb[:], rz1[:])

            # ---- phase B: per-example u (attn) -> cav -> v2t, pipelined;
            # fin1 and pairwise l2 waves fill the mt-DMA wait gaps ----
            psus = [None] * BL

            def u_pass(b):
                psu = psupool.tile([4, 256], F32, tag="psu", name=f"psu{b}")
                for k in range(32):
                    off = 2048 * (k // 8) + 256 * (k % 8)
                    nc.tensor.matmul(
                        psu[:],
                        exp1T[:, 128 * k + 32 * b : 128 * k + 32 * b + 4],
                        mt_sb[b][:, off : off + 256],
                        start=(k == 0),
                        stop=(k == 31),
                    )
                psus[b] = psu

            def cav(b):
                u_pack = spool.tile(
                    [4, 256], F32R, tag="u_pack", bufs=2, name=f"up{b}"
                )
                nc.vector.tensor_scalar_mul(
                    u_pack[:], psus[b][:], rz1[32 * b : 32 * b + 4, :]
                )
                for h in range(2):
                    pstu = pstpool.tile([128, 4], F32R, tag="pst", name=f"pu{b}{h}")
                    nc.tensor.transpose(
                        pstu[:], u_pack[:, 128 * h : 128 * h + 128], eye4_sb[:]
                    )
                    nc.vector.tensor_copy(
                        uT[:, 8 * b + 4 * h : 8 * b + 4 * h + 4], pstu[:]
                    )
                for dcm in range(2):
                    cps = smpool.tile([128, 4], F32, tag="sm", name=f"cv{b}{dcm}")
                    for h in range(2):
                        nc.tensor.matmul(
                            cps[:],
                            w7b_sb[h][:, 128 * dcm : 128 * dcm + 128],
                            uT[:, 8 * b + 4 * h : 8 * b + 4 * h + 4],
                            start=(h == 0),
                            stop=(h == 1),
                        )
                    nc.vector.tensor_add(
                        v2tp[
                            :, 128 * dcm + 32 * b : 128 * dcm + 32 * b + 4
                        ],
                        cps[:],
                        v1t_sb[:, 16 * dcm + 4 * b : 16 * dcm + 4 * b + 4],
                    )

            def l2_pair(p, groups):
                # examples 2p, 2p+1 -> psum rows 32i; act covers 64 rows
                for g in groups:
                    pl2 = megapool.tile(
                        [64, 512], F32, tag="mega", name=f"wc{p}_{g}"
                    )
                    for dc in range(2):
                        for i in range(2):
                            b = 2 * p + i
                            nc.tensor.matmul(
                                pl2[32 * i : 32 * i + 32, :],
                                v2tp[
                                    :,
                                    128 * dc + 32 * b : 128 * dc + 32 * b + 32,
                                ],
                                m_slice(b, dc, g),
                                start=(dc == 0),
                                stop=(dc == 1),
                                tile_position=(0, 32 * i),
                            )
                    nc.scalar.activation(
                        exp2_sb[64 * p : 64 * p + 64, 512 * g : 512 * g + 512],
                        pl2[:],
                        EXP,
                        accum_out=zp2c[64 * p : 64 * p + 64, g : g + 1],
                    )

            def fin_out(which, exp_sb, bd, o_d):
                fin = megapool.tile([32, 512], F32, tag="mega", name=f"fin{which}")
                for g in range(NG):
                    nc.tensor.matmul(
                        fin[:],
                        bd[:, 32 * g : 32 * g + 32],
                        exp_sb[:, 512 * g : 512 * g + 512],
                        start=(g == 0),
                        stop=(g == NG - 1),
                    )
                o_sb = spool.tile([32, 512], F32, tag=f"o{which}", name=f"o{which}")
                nc.scalar.activation(o_sb[:], fin[:], LOG, scale=1.0 / PL)
                o_v = o_d.rearrange("b (g c) -> (b g) c", g=NG)
                nc.scalar.dma_start(out=o_v[:, :], in_=o_sb[:])

            u_pass(0)
            cav(0)
            fin_out(0, exp1_sb, bd31, o1_d)
            u_pass(1)
            cav(1)
            l2_pair(0, range(0, 4))
            u_pass(2)
            cav(2)
            l2_pair(0, range(4, 8))
            u_pass(3)
            cav(3)
            l2_pair(1, range(0, 8))

            # ---- phase C: softmax2 normalizers + final p2 ----
            z2 = spool.tile([128, 1], F32, tag="z2")
            nc.vector.tensor_reduce(
                z2[:], zp2c[:], axis=mybir.AxisListType.X, op=mybir.AluOpType.add
            )
            rz2 = spool.tile([128, 1], F32, tag="rz2")
            nc.vector.reciprocal(rz2[:], z2[:])
            bd32 = spool.tile([128, 256], BF16, tag="bd32")
            nc.vector.tensor_scalar_mul(bd32[:], p4g_sb[:], rz2[:])
            fin_out(1, exp2_sb, bd32, o2_d)

    nc.compile()
    return nc


def _host_precompute(inp):
    H_q, M, W_4, W_6, W_7 = (
        inp["H_q"],
        inp["M"],
        inp["W_4"],
        inp["W_6"],
        inp["W_7"],
    )
    wih, whh, bih, bhh = (
        inp["gru_w_ih"],
        inp["gru_w_hh"],
        inp["gru_b_ih"],
        inp["gru_b_hh"],
    )
    lg = H_q @ W_4
    a = np.exp(lg - lg.max(1, keepdims=True))
    a /= a.sum(1, keepdims=True)
    s = np.einsum("bq,bqh->bh", a, H_q).astype(np.float32)
    x = M.mean(axis=2)
    gh = x @ whh.T + bhh
    ghr, ghz, ghn = np.split(gh, 3, axis=1)
    s_all = [s]
    for _ in range(T - 1):
        gi = s @ wih.T + bih
        gir, giz, gin = np.split(gi, 3, axis=1)
        r = 1.0 / (1.0 + np.exp(-(gir + ghr)))
        z = 1.0 / (1.0 + np.exp(-(giz + ghz)))
        n = np.tanh(gin + r * ghn)
        s = (1.0 - z) * n + z * x
        s_all.append(s)
    S = np.stack(s_all).astype(np.float32)  # (T, B, D2)
    SW6 = np.einsum("tbd,de->tbe", S, W_6).astype(np.float32)
    W7t, W7b = W_7[:D2], W_7[D2:]
    V1 = np.einsum("tbd,de->tbe", S, W7t).astype(np.float32)
    return SW6, V1, W7b


def kernel(**inputs):
    global _NC
    inp = {
        k: np.ascontiguousarray(np.asarray(v, dtype=np.float32))
        for k, v in inputs.items()
    }
    SW6, V1, W7b = _host_precompute(inp)
    M = inp["M"]

    eye = np.eye(128, dtype=np.float32).astype(ml_dtypes.bfloat16)
    eye4 = np.eye(4, dtype=np.float32)
    # p4g[:, 32g:32g+32][32b+t, 8b'+g'] = d(b=b') * d(g=g')  (t < 4 rows only)
    rows = np.arange(128)
    bb, tt = rows // 32, rows % 32
    p4g = np.zeros((128, 256), dtype=np.float32)
    valid = tt < 4
    for g in range(NG):
        p4g[rows[valid], 32 * g + 8 * bb[valid] + g] = 1.0
    p4g = p4g.astype(ml_dtypes.bfloat16)
    w7b_h = np.ascontiguousarray(W7b.reshape(2, 128, 256))

    if _NC is None:
        _NC = _build_graph()
    in_maps = []
    for i in range(NCORES):
        sl = slice(i * BL, (i + 1) * BL)
        Mc = M[sl]  # (BL, 256, PL)
        # m_h[q][p, 2048b + 1024dc + c] = Mc[b, 128dc + p, 1024q + c]
        m_h = np.ascontiguousarray(
            Mc.reshape(BL, 2, 128, 4, 1024)
            .transpose(3, 2, 0, 1, 4)
            .reshape(4, 128, 8192)
            .astype(np.float16)
        )
        # mt_h[b][p, 2048kg + 256kk + e] = Mc[b, e, 1024kg + 128kk + p]
        mt_h = np.ascontiguousarray(
            Mc.transpose(0, 2, 1)
            .reshape(BL, 4, 8, 128, 256)
            .transpose(0, 3, 1, 2, 4)
            .reshape(BL, 128, 8192)
            .astype(ml_dtypes.bfloat16)
        )
        # sw6p[p, 128dc + 32b + j] = SW6[j, b, 128dc + p] (j < 4, else 0)
        s_ = SW6[:, sl].reshape(T, BL, 2, 128)  # t, b, dc, p
        sw6p = np.zeros((128, 2, BL, 32), np.float32)
        sw6p[:, :, :, :T] = s_.transpose(3, 2, 1, 0)
        sw6p = np.ascontiguousarray(
            sw6p.reshape(128, 256).astype(np.float16)
        )
        # v1t[dc][p, 4b + t] = V1[t, b, 128dc + p]
        v1c = np.ascontiguousarray(
            V1[:, sl].transpose(2, 1, 0).reshape(2, 128, 16).astype(np.float16)
        )
        in_maps.append(
            {
                "m": m_h,
                "mt": mt_h,
                "sw6p": sw6p,
                "v1t": v1c,
                "w7b": w7b_h,
                "eye": eye,
                "eye4": eye4,
                "p4g": p4g,
            }
        )
    global _LAST_IN_MAPS
    _LAST_IN_MAPS = in_maps
    res = run_bass_kernel_spmd(_NC, in_maps, core_ids=list(range(NCORES)))
    out1 = np.empty((B, PL), np.float32)
    out2 = np.empty((B, PL), np.float32)
    for i in range(NCORES):
        out1[i * BL : (i + 1) * BL] = res.results[i]["o1"]
        out2[i * BL : (i + 1) * BL] = res.results[i]["o2"]
    return out1, out2


# revision 9
# speedup vs baseline: 1.0722x; 1.0722x over previous
"""Trainium2 Bass kernel for nn_AnswerModule (scatter_memory, 8 cores).

Strategy: pure data-parallel over batch (4 examples per core).  The
heavy einsums are collapsed via matmul associativity:
  p1 = softmax((s@W6) @ M),  attn@W7b = p1 @ (M^T @ W7b),
  p2 = softmax((s@W7t + attn@W7b) @ M).
The tiny GRU / alpha-attention recurrence runs on host.

This revision (vs the first working kernel) targets three measured
bottlenecks:
  1. DMA descriptor-processing bound (input stream took ~78us at 4-8KB
     descriptors): m/mt now land as 8 fat DMAs with 16KB contiguous
     runs per partition, reaching the ~45us byte-bound floor.
  2. Scalar engine at 48us: EXP activations ran on (4,512) tiles using
     4/128 partitions.  Logit waves for all 4 examples now target one
     (128,512) psum tile via zero-padded 32-col stationaries (junk rows
     compute exp(0)=1, matching the old memset semantics), so each EXP
     act covers 128 rows.  accum_out on the same act yields the softmax
     Z-partials for free (removes the DVE reduy reduces).
  3. PE stalls/p-state: one long phase-ordered PE stream (l1 waves ->
     transposes -> u/cav pipeline with fin1 + pairwise l2 waves filling
     the mt-DMA gaps -> l2 tail -> fin2).
"""

import sys

sys.path.insert(0, "/opt/trn_rl_repo")

import numpy as np
import ml_dtypes

import concourse.bass as bass
import concourse.bacc as bacc
import concourse.mybir as mybir
from concourse import tile
from concourse.bass_utils import run_bass_kernel_spmd

B, QL, PL, T, D2 = 32, 64, 4096, 4, 256
NCORES = 8
BL = B // NCORES  # 4 examples per core
NG = 8  # n-groups of 512
F32 = mybir.dt.float32
F32R = mybir.dt.float32r
BF16 = mybir.dt.bfloat16
FP16 = mybir.dt.float16

_NC = None


def _build_graph():
    nc = bacc.Bacc("TRN2", target_bir_lowering=False, debug=False)

    # DRAM inputs.  m: quarter q covers n-groups 2q,2q+1; col layout
    # 2048*b + 1024*dc + c.  mt: per-example M^T, col 2048*kg + 256*kk + e.
    m_d = nc.dram_tensor("m", [4, 128, 8192], FP16, kind="ExternalInput").ap()
    mt_d = nc.dram_tensor("mt", [BL, 128, 8192], BF16, kind="ExternalInput").ap()
    sw6p_d = nc.dram_tensor("sw6p", [128, 256], FP16, kind="ExternalInput").ap()
    v1t_d = nc.dram_tensor("v1t", [2, 128, 16], FP16, kind="ExternalInput").ap()
    w7b_d = nc.dram_tensor("w7b", [2, 128, 256], F32R, kind="ExternalInput").ap()
    eye_d = nc.dram_tensor("eye", [128, 128], BF16, kind="ExternalInput").ap()
    eye4_d = nc.dram_tensor("eye4", [4, 4], F32R, kind="ExternalInput").ap()
    p4g_d = nc.dram_tensor("p4g", [128, 256], BF16, kind="ExternalInput").ap()
    o1_d = nc.dram_tensor("o1", [BL, PL], F32, kind="ExternalOutput").ap()
    o2_d = nc.dram_tensor("o2", [BL, PL], F32, kind="ExternalOutput").ap()

    EXP = mybir.ActivationFunctionType.Exp
    LOG = getattr(mybir.ActivationFunctionType, "Log", None) or getattr(
        mybir.ActivationFunctionType, "Ln"
    )

    with tile.TileContext(nc) as tc:
        with (
            nc.allow_low_precision(reason="float32r is 4-byte, same width as f32"),
            tc.tile_pool(name="const", bufs=1) as cpool,
            tc.tile_pool(name="m", bufs=1) as mpool,
            tc.tile_pool(name="mt", bufs=1) as mtpool,
            tc.tile_pool(name="sb", bufs=1) as spool,
            tc.tile_pool(name="mega", bufs=3, space="PSUM") as megapool,
            tc.tile_pool(name="pst", bufs=2, space="PSUM") as pstpool,
            tc.tile_pool(name="psu", bufs=2, space="PSUM") as psupool,
            tc.tile_pool(name="sm", bufs=1, space="PSUM") as smpool,
        ):
            # ---- SBUF tiles ----
            sw6p_sb = cpool.tile([128, 256], FP16, tag="sw6p")
            v1t_sb = cpool.tile([128, 32], FP16, tag="v1t")
            w7b_sb = []
            eye_sb = cpool.tile([128, 128], BF16, tag="eye")
            eye4_sb = cpool.tile([4, 4], F32R, tag="eye4")
            p4g_sb = cpool.tile([128, 256], BF16, tag="p4g")
            m_sb = mpool.tile([128, 32768], FP16, tag="m")
            mt_sb = [
                mtpool.tile([128, 8192], BF16, tag="mt", name=f"mt{b}")
                for b in range(BL)
            ]
            exp1_sb = spool.tile([128, PL], BF16, tag="exp1")
            exp2_sb = spool.tile([128, PL], BF16, tag="exp2")
            exp1T = spool.tile([128, PL], BF16, tag="exp1T")
            zp1c = spool.tile([128, NG], F32, tag="zp1c")
            zp2c = spool.tile([128, NG], F32, tag="zp2c")
            v2tp = spool.tile([128, 256], FP16, tag="v2tp")
            uT = spool.tile([128, 32], F32R, tag="uT")

            # ---- small-const DMAs on the scalar HWDGE queue (parallel to
            # the payload queue) except sw6p/eye which ride the sync queue
            # early.  LOG table preload keys off sw6p so it fires ~0.2us in;
            # scale=0 + bias=1 keeps the junk result finite (log(1)=0).
            for dc in range(2):
                t_ = cpool.tile([128, 256], F32R, tag=f"w7b{dc}", name=f"w7b{dc}")
                nc.scalar.dma_start(out=t_[:], in_=w7b_d[dc])
                w7b_sb.append(t_)
            nc.scalar.dma_start(out=p4g_sb[:], in_=p4g_d[:, :])
            nc.scalar.dma_start(out=eye4_sb[:], in_=eye4_d[:, :])
            for dc in range(2):
                nc.scalar.dma_start(
                    out=v1t_sb[:, 16 * dc : 16 * dc + 16], in_=v1t_d[dc]
                )

            # ---- m payload DMAs on the sync (SP) queue, priority order.
            # mt rides the scalar HWDGE queue (own ring) and is issued inside
            # the scalar stream after the late l1 EXPs, so it starts streaming
            # right as m drains without competing for bandwidth earlier.
            nc.sync.dma_start(out=sw6p_sb[:], in_=sw6p_d[:, :])
            nc.sync.dma_start(out=eye_sb[:], in_=eye_d[:, :])
            for q in range(4):
                nc.sync.dma_start(
                    out=m_sb[:, 8192 * q : 8192 * q + 8192], in_=m_d[q]
                )

            nc.gpsimd.memset(v2tp[:], 0.0)

            # mt half-DMAs, issued later: halves (b, h) -> scalar queue
            def mt_dma(b, h):
                nc.scalar.dma_start(
                    out=mt_sb[b][:, 4096 * h : 4096 * h + 4096],
                    in_=mt_d[b, :, 4096 * h : 4096 * h + 4096],
                )

            def m_slice(b, dc, g):
                off = 8192 * (g // 2) + 2048 * b + 1024 * dc + 512 * (g % 2)
                return m_sb[:, off : off + 512]

            # ---- phase A: l1 logit waves, dense EXP evac + Z partials ----
            def transpose_chunk(k):
                pst = pstpool.tile([128, 128], BF16, tag="pst", name=f"pst{k}")
                nc.tensor.transpose(
                    pst[:], exp1_sb[:, 128 * k : 128 * k + 128], eye_sb[:]
                )
                nc.vector.tensor_copy(exp1T[:, 128 * k : 128 * k + 128], pst[:])

            # mt halves issued on the scalar stream after late-phase exps:
            # (group after which to issue) -> list of (b, h)
            mt_sched = {
                5: [(0, 0)],
                6: [(0, 1), (1, 0)],
                7: [(1, 1), (2, 0), (2, 1), (3, 0), (3, 1)],
            }
            for g in range(NG):
                mega = megapool.tile([128, 512], F32, tag="mega", name=f"wa{g}")
                for dc in range(2):
                    for b in range(BL):
                        nc.tensor.matmul(
                            mega[32 * b : 32 * b + 32, :],
                            sw6p_sb[:, 128 * dc + 32 * b : 128 * dc + 32 * b + 32],
                            m_slice(b, dc, g),
                            start=(dc == 0),
                            stop=(dc == 1),
                            tile_position=(0, 32 * b),
                        )
                nc.scalar.activation(
                    exp1_sb[:, 512 * g : 512 * g + 512],
                    mega[:],
                    EXP,
                    accum_out=zp1c[:, g : g + 1],
                )
                for bh in mt_sched.get(g, ()):
                    mt_dma(*bh)
                if g > 0:
                    for j in range(4):
                        transpose_chunk(4 * (g - 1) + j)
            for j in range(4):
                transpose_chunk(28 + j)

            # softmax1 normalizers
            z1 = spool.tile([128, 1], F32, tag="z1")
            nc.vector.tensor_reduce(
                z1[:], zp1c[:], axis=mybir.AxisListType.X, op=mybir.AluOpType.add
            )
            rz1 = spool.tile([128, 1], F32, tag="rz1")
            nc.vector.reciprocal(rz1[:], z1[:])
            bd31 = spool.tile([128, 256], BF16, tag="bd31")
            nc.vector.tensor_scalar_mul(bd31[:], p4g_sb[:], rz1[:])

            # ---- phase B: per-example u (attn) -> cav -> v2t, pipelined;
            # fin1 and pairwise l2 waves fill the mt-DMA wait gaps ----
            psus = [None] * BL

            def u_pass(b):
                psu = psupool.tile([4, 256], F32, tag="psu", name=f"psu{b}")
                for k in range(32):
                    off = 2048 * (k // 8) + 256 * (k % 8)
                    nc.tensor.matmul(
                        psu[:],
                        exp1T[:, 128 * k + 32 * b : 128 * k + 32 * b + 4],
                        mt_sb[b][:, off : off + 256],
                        start=(k == 0),
                        stop=(k == 31),
                    )
                psus[b] = psu

            def cav(b):
                u_pack = spool.tile(
                    [4, 256], F32R, tag="u_pack", bufs=2, name=f"up{b}"
                )
                nc.vector.tensor_scalar_mul(
                    u_pack[:], psus[b][:], rz1[32 * b : 32 * b + 4, :]
                )
                for h in range(2):
                    pstu = pstpool.tile([128, 4], F32R, tag="pst", name=f"pu{b}{h}")
                    nc.tensor.transpose(
                        pstu[:], u_pack[:, 128 * h : 128 * h + 128], eye4_sb[:]
                    )
                    nc.vector.tensor_copy(
                        uT[:, 8 * b + 4 * h : 8 * b + 4 * h + 4], pstu[:]
                    )
                for dcm in range(2):
                    cps = smpool.tile([128, 4], F32, tag="sm", name=f"cv{b}{dcm}")
                    for h in range(2):
                        nc.tensor.matmul(
                            cps[:],
                            w7b_sb[h][:, 128 * dcm : 128 * dcm + 128],
                            uT[:, 8 * b + 4 * h : 8 * b + 4 * h + 4],
                            start=(h == 0),
                            stop=(h == 1),
                        )
                    nc.vector.tensor_add(
                        v2tp[
                            :, 128 * dcm + 32 * b : 128 * dcm + 32 * b + 4
                        ],
                        cps[:],
                        v1t_sb[:, 16 * dcm + 4 * b : 16 * dcm + 4 * b + 4],
                    )

            def l2_pair(p, groups):
                # examples 2p, 2p+1 -> psum rows 32i; act covers 64 rows
                for g in groups:
                    pl2 = megapool.tile(
                        [64, 512], F32, tag="mega", name=f"wc{p}_{g}"
                    )
                    for dc in range(2):
                        for i in range(2):
                            b = 2 * p + i
                            nc.tensor.matmul(
                                pl2[32 * i : 32 * i + 32, :],
                                v2tp[
                                    :,
                                    128 * dc + 32 * b : 128 * dc + 32 * b + 32,
                                ],
                                m_slice(b, dc, g),
                                start=(dc == 0),
                                stop=(dc == 1),
                                tile_position=(0, 32 * i),
                            )
                    nc.scalar.activation(
                        exp2_sb[64 * p : 64 * p + 64, 512 * g : 512 * g + 512],
                        pl2[:],
                        EXP,
                        accum_out=zp2c[64 * p : 64 * p + 64, g : g + 1],
                    )

            def fin_mm(which, exp_sb, bd):
                # final p matmuls; result parked in SBUF (f32) so both LOG
                # activations can run back-to-back at the end (one act-table
                # load instead of EXP<->LOG thrash).
                fin = megapool.tile([32, 512], F32, tag="mega", name=f"fin{which}")
                for g in range(NG):
                    nc.tensor.matmul(
                        fin[:],
                        bd[:, 32 * g : 32 * g + 32],
                        exp_sb[:, 512 * g : 512 * g + 512],
                        start=(g == 0),
                        stop=(g == NG - 1),
                    )
                f_sb = spool.tile([32, 512], F32, tag=f"f{which}", name=f"f{which}")
                nc.vector.tensor_copy(f_sb[:], fin[:])
                return f_sb

            def log_out(which, f_sb, o_d):
                o_sb = spool.tile([32, 512], F32, tag=f"o{which}", name=f"o{which}")
                nc.scalar.activation(o_sb[:], f_sb[:], LOG, scale=1.0 / PL)
                o_v = o_d.rearrange("b (g c) -> (b g) c", g=NG)
                nc.sync.dma_start(out=o_v[:, :], in_=o_sb[:])

            u_pass(0)
            cav(0)
            f1_sb = fin_mm(0, exp1_sb, bd31)
            u_pass(1)
            cav(1)
            l2_pair(0, range(0, 4))
            u_pass(2)
            cav(2)
            l2_pair(0, range(4, 8))
            u_pass(3)
            cav(3)
            l2_pair(1, range(0, 8))

            # ---- phase C: softmax2 normalizers + final p2 ----
            z2 = spool.tile([128, 1], F32, tag="z2")
            nc.vector.tensor_reduce(
                z2[:], zp2c[:], axis=mybir.AxisListType.X, op=mybir.AluOpType.add
            )
            rz2 = spool.tile([128, 1], F32, tag="rz2")
            nc.vector.reciprocal(rz2[:], z2[:])
            bd32 = spool.tile([128, 256], BF16, tag="bd32")
            nc.vector.tensor_scalar_mul(bd32[:], p4g_sb[:], rz2[:])
            f2_sb = fin_mm(1, exp2_sb, bd32)
            log_out(0, f1_sb, o1_d)
            log_out(1, f2_sb, o2_d)

    nc.compile()
    return nc


def _host_precompute(inp):
    H_q, M, W_4, W_6, W_7 = (
        inp["H_q"],
        inp["M"],
        inp["W_4"],
        inp["W_6"],
        inp["W_7"],
    )
    wih, whh, bih, bhh = (
        inp["gru_w_ih"],
        inp["gru_w_hh"],
        inp["gru_b_ih"],
        inp["gru_b_hh"],
    )
    lg = H_q @ W_4
    a = np.exp(lg - lg.max(1, keepdims=True))
    a /= a.sum(1, keepdims=True)
    s = np.einsum("bq,bqh->bh", a, H_q).astype(np.float32)
    x = M.mean(axis=2)
    gh = x @ whh.T + bhh
    ghr, ghz, ghn = np.split(gh, 3, axis=1)
    s_all = [s]
    for _ in range(T - 1):
        gi = s @ wih.T + bih
        gir, giz, gin = np.split(gi, 3, axis=1)
        r = 1.0 / (1.0 + np.exp(-(gir + ghr)))
        z = 1.0 / (1.0 + np.exp(-(giz + ghz)))
        n = np.tanh(gin + r * ghn)
        s = (1.0 - z) * n + z * x
        s_all.append(s)
    S = np.stack(s_all).astype(np.float32)  # (T, B, D2)
    SW6 = np.einsum("tbd,de->tbe", S, W_6).astype(np.float32)
    W7t, W7b = W_7[:D2], W_7[D2:]
    V1 = np.einsum("tbd,de->tbe", S, W7t).astype(np.float32)
    return SW6, V1, W7b


def kernel(**inputs):
    global _NC
    inp = {
        k: np.ascontiguousarray(np.asarray(v, dtype=np.float32))
        for k, v in inputs.items()
    }
    SW6, V1, W7b = _host_precompute(inp)
    M = inp["M"]

    eye = np.eye(128, dtype=np.float32).astype(ml_dtypes.bfloat16)
    eye4 = np.eye(4, dtype=np.float32)
    # p4g[:, 32g:32g+32][32b+t, 8b'+g'] = d(b=b') * d(g=g')  (t < 4 rows only)
    rows = np.arange(128)
    bb, tt = rows // 32, rows % 32
    p4g = np.zeros((128, 256), dtype=np.float32)
    valid = tt < 4
    for g in range(NG):
        p4g[rows[valid], 32 * g + 8 * bb[valid] + g] = 1.0
    p4g = p4g.astype(ml_dtypes.bfloat16)
    w7b_h = np.ascontiguousarray(W7b.reshape(2, 128, 256))

    if _NC is None:
        _NC = _build_graph()
    in_maps = []
    for i in range(NCORES):
        sl = slice(i * BL, (i + 1) * BL)
        Mc = M[sl]  # (BL, 256, PL)
        # m_h[q][p, 2048b + 1024dc + c] = Mc[b, 128dc + p, 1024q + c]
        m_h = np.ascontiguousarray(
            Mc.reshape(BL, 2, 128, 4, 1024)
            .transpose(3, 2, 0, 1, 4)
            .reshape(4, 128, 8192)
            .astype(np.float16)
        )
        # mt_h[b][p, 2048kg + 256kk + e] = Mc[b, e, 1024kg + 128kk + p]
        mt_h = np.ascontiguousarray(
            Mc.transpose(0, 2, 1)
            .reshape(BL, 4, 8, 128, 256)
            .transpose(0, 3, 1, 2, 4)
            .reshape(BL, 128, 8192)
            .astype(ml_dtypes.bfloat16)
        )
        # sw6p[p, 128dc + 32b + j] = SW6[j, b, 128dc + p] (j < 4, else 0)
        s_ = SW6[:, sl].reshape(T, BL, 2, 128)  # t, b, dc, p
        sw6p = np.zeros((128, 2, BL, 32), np.float32)
        sw6p[:, :, :, :T] = s_.transpose(3, 2, 1, 0)
        sw6p = np.ascontiguousarray(
            sw6p.reshape(128, 256).astype(np.float16)
        )
        # v1t[dc][p, 4b + t] = V1[t, b, 128dc + p]
        v1c = np.ascontiguousarray(
            V1[:, sl].transpose(2, 1, 0).reshape(2, 128, 16).astype(np.float16)
        )
        in_maps.append(
            {
                "m": m_h,
                "mt": mt_h,
                "sw6p": sw6p,
                "v1t": v1c,
                "w7b": w7b_h,
                "eye": eye,
                "eye4": eye4,
                "p4g": p4g,
            }
        )
    global _LAST_IN_MAPS
    _LAST_IN_MAPS = in_maps
    res = run_bass_kernel_spmd(_NC, in_maps, core_ids=list(range(NCORES)))
    out1 = np.empty((B, PL), np.float32)
    out2 = np.empty((B, PL), np.float32)
    for i in range(NCORES):
        out1[i * BL : (i + 1) * BL] = res.results[i]["o1"]
        out2[i * BL : (i + 1) * BL] = res.results[i]["o2"]
    return out1, out2
